# revision 15
# baseline (speedup 1.0000x reference)
"""DGD memory-update kernel for Trainium2 (8 NeuronCores, SPMD).

Computes, per (b, h) pair:
    pred = M @ k                    # [D,1] matvec
    err  = pred - v
    out  = alpha * M + eta * err @ k^T

Sharding: batch dim B=16 split across 8 cores (2 b x 16 h = 32 pairs/core).
Zero inter-core communication.

Layout: D=512 rows live as 4 row-chunks on 128 partitions, partition-major
(SBUF tile[p, c*512+j] = M[4*p+c, j]) so each partition's DMA run is 8KB
contiguous.  Pairs are processed in groups of G=2 adjacent h so every DMA /
shuffle / elementwise pass is issued once per group — per-instruction issue
overhead (~1-3us per DMA on the issuing sequencer) is the limiter, not
bandwidth.

Per-group dataflow:
  - k broadcast: one strided DMA plants both pairs' k at partitions
    {0,32,64,96}, one DVE stream_shuffle(mask=[0]*32) replicates lane 0 of
    each 32-group (custom gpsimd/PE broadcast paths are broken on this
    stack; this is exact and HW-verified).
  - DVE tensor_tensor mult (batched [128,G,4,512], k broadcast along the
    chunk axis via a stride-0 AP) -> product, written into the out tile
    (dead until the final add overwrites it).
  - ACT activation(Copy, accum_out) row-reduces the product per chunk ->
    pred (free-axis accumulate on the scalar engine).
  - s_err = eta*(pred - v) on [128,G*4] (tiny DVE ops).
  - ACT pre-scales Ms = alpha*M per pair (per-partition scale AP).
  - DVE tensor_scalar builds outer[:,q,c,:] = k_bc * s_err[:,q,c].
  - GPSIMD tensor_tensor add (one batched op): out = Ms + outer.
  - M-in DMA rides gpsimd's SWDGE, out DMA rides SP's HWDGE (split so no
    single sequencer saturates).
HBM traffic is the 2MB/pair floor (M in + out) -> DMA-bound target ~6us/pair.
"""

import numpy as np

B, H, D = 16, 16, 512
N_CORES = 8
B_PER_CORE = B // N_CORES            # 2
PAIRS_PER_CORE = B_PER_CORE * H      # 32
P = 128                              # SBUF partitions
C = D // P                           # 4 row-chunks per pair
G = 1                                # pairs per group
NG = PAIRS_PER_CORE // G             # 16 groups

_CACHE = {}


def _legalize_sync_waits(nc, mybir, max_waits=1):
    # The walrus build in this container rejects instructions carrying more
    # than one sync wait; hoist excess waits onto preceding same-engine NOPs.
    for f in nc.m.functions:
        for bb in f.blocks:
            out = []
            for inst in bb.instructions:
                si = inst.sync_info
                if si is not None and si.on_wait and len(si.on_wait) > max_waits:
                    waits = list(si.on_wait)
                    excess, keep = waits[:-max_waits], waits[-max_waits:]
                    for i in range(0, len(excess), max_waits):
                        nop = mybir.InstNoOp(
                            name=nc.get_next_instruction_name(),
                            engine=inst.engine,
                            ins=[],
                            outs=[],
                            bass_nofuse=True,
                            sync_info=mybir.SyncInfo(
                                on_wait=excess[i : i + max_waits], on_update=[]
                            ),
                        )
                        nc.register_instruction(nop)
                        out.append(nop)
                    si.on_wait = keep
                out.append(inst)
            bb.instructions[:] = out


def _build_program():
    import concourse.bass as bass
    import concourse.tile as tile
    from concourse import mybir

    f32 = mybir.dt.float32
    mult = mybir.AluOpType.mult
    add = mybir.AluOpType.add
    subtract = mybir.AluOpType.subtract
    Copy = mybir.ActivationFunctionType.Copy
    BCAST32 = [0] * 32  # stream_shuffle: every lane reads lane 0 of its 32-group

    nc = bass.Bass()
    mem_ext = nc.dram_tensor("memory", [B_PER_CORE, H, D, D], f32, kind="ExternalInput")
    k_ext = nc.dram_tensor("k", [B_PER_CORE, H, D, 1], f32, kind="ExternalInput")
    v_ext = nc.dram_tensor("v", [B_PER_CORE, H, D, 1], f32, kind="ExternalInput")
    alpha_ext = nc.dram_tensor("alpha", [B_PER_CORE, H, 1, 1], f32, kind="ExternalInput")
    eta_ext = nc.dram_tensor("eta", [B_PER_CORE, H, 1, 1], f32, kind="ExternalInput")
    out_ext = nc.dram_tensor("out", [B_PER_CORE, H, D, D], f32, kind="ExternalOutput")

    NP = PAIRS_PER_CORE
    GD = G * C * D  # free elems per group tile (4096)
    p4 = slice(0, P, 32)  # partitions {0,32,64,96}

    with tile.TileContext(nc) as tc:
        with (
            tc.tile_pool(name="const", bufs=1) as const_pool,
            tc.tile_pool(name="m_in", bufs=4) as m_pool,
            tc.tile_pool(name="ms", bufs=2) as ms_pool,
            tc.tile_pool(name="outer", bufs=2) as outer_pool,
            tc.tile_pool(name="outt", bufs=3) as out_pool,
            tc.tile_pool(name="kbc", bufs=3) as kbc_pool,
            tc.tile_pool(name="scratch", bufs=2) as scratch_pool,
            tc.tile_pool(name="tmpp", bufs=2) as tmp_pool,
            tc.tile_pool(name="small", bufs=3) as small_pool,
        ):
            # --- one-time: broadcast alpha (cols 0:NP) and eta (cols NP:2NP) ---
            ab4 = const_pool.tile([P, 2 * NP], f32)
            nc.vector.memset(ab4[:], 0.0)
            nc.sync.dma_start(
                ab4[p4, 0:NP],
                alpha_ext[:].flatten().rearrange("(o c) -> o c", o=1).broadcast_to((4, NP)),
            )
            nc.sync.dma_start(
                ab4[p4, NP : 2 * NP],
                eta_ext[:].flatten().rearrange("(o c) -> o c", o=1).broadcast_to((4, NP)),
            )
            ab_bc = const_pool.tile([P, 2 * NP], f32)
            nc.vector.stream_shuffle(ab_bc[:], ab4[:], BCAST32)

            # Long-lived ping-pong landing pads for k: the per-group DMA
            # writes only partitions {0,32,64,96}; the shuffle reads a full
            # [128] AP, so the tiles must stay initialized across groups.
            kb4_tiles = []
            for i in range(2):
                t = const_pool.tile([P, G * D], f32, tag=f"kb4_{i}")
                nc.vector.memset(t[:], 0.0)
                kb4_tiles.append(t)

            # --- main loop over groups of G adjacent pairs ---
            for g in range(NG):
                p0 = g * G
                b, h = divmod(p0, H)  # pairs (b, h) .. (b, h+G-1)

                m_in = m_pool.tile([P, GD], f32)
                # SWDGE path: SP alone saturates issuing all DMAs
                # flat AP: each partition's 8KB is contiguous in DRAM ->
                # 1 descriptor per partition instead of 4
                nc.gpsimd.dma_start(
                    m_in[:],
                    mem_ext[b, h : h + G].flatten().rearrange("(p x) -> p x", p=P),
                )
                kb4 = kb4_tiles[g % 2]
                nc.sync.dma_start(
                    kb4[p4, :],
                    k_ext[b, h : h + G]
                    .flatten()
                    .rearrange("(o x) -> o x", o=1)
                    .broadcast_to((4, G * D)),
                )
                v_pc = small_pool.tile([P, G * C], f32, tag="v_pc")
                nc.sync.dma_start(
                    v_pc[:].rearrange("p (q c) -> p q c", q=G),
                    v_ext[b, h : h + G].flatten().rearrange("(q p c) -> p q c", q=G, p=P),
                )

                # k broadcast to all partitions: k_bc[p, q*D+j] = k_q[j]
                k_bc = kbc_pool.tile([P, G * D], f32)
                nc.vector.stream_shuffle(k_bc[:], kb4[:], BCAST32)

                # Ms = alpha * M  (per pair: per-partition scale AP).
                # Chunks 0-2 on ACT, chunk 3 on DVE (tensor_scalar runs in
                # 2x mode) to balance the two engines.
                ms = ms_pool.tile([P, GD], f32)
                for q in range(G):
                    base = q * C * D
                    nc.scalar.activation(
                        ms[:, base : base + 3 * D],
                        m_in[:, base : base + 3 * D],
                        Copy,
                        scale=ab_bc[:, p0 + q : p0 + q + 1],
                    )
                    nc.vector.tensor_scalar_mul(
                        ms[:, base + 3 * D : base + 4 * D],
                        m_in[:, base + 3 * D : base + 4 * D],
                        ab_bc[:, p0 + q : p0 + q + 1],
                    )

                # product = M (*) k, one batched op
                tmp = tmp_pool.tile([P, GD], f32)
                nc.vector.tensor_tensor(
                    tmp[:].rearrange("p (q c j) -> p q c j", q=G, c=C),
                    m_in[:].rearrange("p (q c j) -> p q c j", q=G, c=C),
                    k_bc[:]
                    .rearrange("p (q o j) -> p q o j", q=G, o=1)
                    .broadcast_to((P, G, C, D)),
                    mult,
                )

                # pred[:, q*C+c] = row-sum of product chunk (ACT accumulate)
                pred = small_pool.tile([P, G * C], f32, tag="pred")
                trash = scratch_pool.tile([P, D], f32, tag="trash")
                for qc in range(G * C):
                    nc.scalar.activation(
                        trash[:],
                        tmp[:, qc * D : (qc + 1) * D],
                        Copy,
                        accum_out=pred[:, qc : qc + 1],
                    )

                # s_err = eta * (pred - v)
                terr = small_pool.tile([P, G * C], f32, tag="terr")
                nc.vector.tensor_tensor(terr[:], pred[:], v_pc[:], subtract)
                s_err = small_pool.tile([P, G * C], f32, tag="s_err")
                for q in range(G):
                    nc.vector.tensor_scalar_mul(
                        s_err[:, q * C : (q + 1) * C],
                        terr[:, q * C : (q + 1) * C],
                        ab_bc[:, NP + p0 + q : NP + p0 + q + 1],
                    )

                # outer[:, q, c, :] = k_bc[:, q, :] * s_err[:, q*C+c]
                outer = outer_pool.tile([P, GD], f32)
                for q in range(G):
                    for c in range(C):
                        nc.vector.tensor_scalar_mul(
                            outer[:, (q * C + c) * D : (q * C + c + 1) * D],
                            k_bc[:, q * D : (q + 1) * D],
                            s_err[:, q * C + c : q * C + c + 1],
                        )

                # out = Ms + outer  (GPSIMD, one batched op)
                out_t = out_pool.tile([P, GD], f32)
                nc.gpsimd.tensor_tensor(out_t[:], ms[:], outer[:], add)

                nc.sync.dma_start(
                    out_ext[b, h : h + G].flatten().rearrange("(p x) -> p x", p=P),
                    out_t[:],
                )

    # Raw Bass (no Bacc.compile) skips the InstISA byte-encoding pass; without
    # it walrus fails with "ISA wrong length" on extended instructions.
    mybir.codegen_inst_isa_subclasses(nc)
    _legalize_sync_waits(nc, mybir)
    return nc


def _get_program():
    if "nc" not in _CACHE:
        _CACHE["nc"] = _build_program()
    return _CACHE["nc"]


def _run(in_maps, **kwargs):
    from concourse.bass_utils import run_bass_kernel_spmd

    nc = _get_program()
    return run_bass_kernel_spmd(nc, in_maps, list(range(N_CORES)), **kwargs)


def _make_in_maps(memory, k, v, alpha, eta):
    def prep(x):
        return np.ascontiguousarray(np.asarray(x, dtype=np.float32))

    memory, k, v, alpha, eta = map(prep, (memory, k, v, alpha, eta))
    in_maps = []
    for i in range(N_CORES):
        s = slice(i * B_PER_CORE, (i + 1) * B_PER_CORE)
        in_maps.append(
            {
                "memory": memory[s],
                "k": k[s],
                "v": v[s],
                "alpha": alpha[s],
                "eta": eta[s],
            }
        )
    return in_maps


def kernel(memory, k, v, alpha, eta):
    res = _run(_make_in_maps(memory, k, v, alpha, eta))
    return np.concatenate(
        [res.results[i]["out"] for i in range(N_CORES)], axis=0
    )


# revision 20
# speedup vs baseline: 1.2119x; 1.2119x over previous
"""DGD memory-update kernel for Trainium2 (8 NeuronCores, SPMD).

Computes, per (b, h) pair:
    pred = M @ k                    # [D,1] matvec
    err  = pred - v
    out  = alpha * M + eta * err @ k^T

Sharding: batch dim B=16 split across 8 cores (2 b x 16 h = 32 pairs/core).
Zero inter-core communication.

Layout: D=512 rows live as 4 row-chunks on 128 partitions, partition-major
(SBUF tile[p, c*512+j] = M[4*p+c, j]) so each partition's DMA run is 8KB
contiguous.  Pairs are processed one per group (G=1; G=2 batching measured
worse end-to-end in the cost model — shallower pipelining outweighed the
issue-overhead savings).

Per-group dataflow:
  - k broadcast: one strided DMA plants both pairs' k at partitions
    {0,32,64,96}, one DVE stream_shuffle(mask=[0]*32) replicates lane 0 of
    each 32-group (custom gpsimd/PE broadcast paths are broken on this
    stack; this is exact and HW-verified).
  - DVE tensor_tensor mult (batched [128,G,4,512], k broadcast along the
    chunk axis via a stride-0 AP) -> product.
  - ACT activation(Copy, accum_out) row-reduces the product per chunk ->
    pred (free-axis accumulate on the scalar engine).
  - s_err = eta*(pred - v) on [128,G*4] (tiny DVE ops).
  - ACT pre-scales Ms = alpha*M per pair (per-partition scale AP).
  - DVE tensor_scalar builds outer[:,q,c,:] = k_bc * s_err[:,q,c].
  - GPSIMD tensor_tensor add (one batched op): out = Ms + outer.
  - M-in DMA rides gpsimd's SWDGE, out DMA rides SP's HWDGE (split so no
    single sequencer saturates).
HBM traffic is the 2MB/pair floor (M in + out) -> DMA-bound target ~6us/pair.
"""

import numpy as np

B, H, D = 16, 16, 512
N_CORES = 8
B_PER_CORE = B // N_CORES            # 2
PAIRS_PER_CORE = B_PER_CORE * H      # 32
P = 128                              # SBUF partitions
C = D // P                           # 4 row-chunks per pair
G = 1                                # pairs per group
NG = PAIRS_PER_CORE // G             # 16 groups

_CACHE = {}


def _legalize_sync_waits(nc, mybir, max_waits=1):
    # The walrus build in this container rejects instructions carrying more
    # than one sync wait; hoist excess waits onto preceding same-engine NOPs.
    for f in nc.m.functions:
        for bb in f.blocks:
            out = []
            for inst in bb.instructions:
                si = inst.sync_info
                if si is not None and si.on_wait and len(si.on_wait) > max_waits:
                    waits = list(si.on_wait)
                    excess, keep = waits[:-max_waits], waits[-max_waits:]
                    for i in range(0, len(excess), max_waits):
                        nop = mybir.InstNoOp(
                            name=nc.get_next_instruction_name(),
                            engine=inst.engine,
                            ins=[],
                            outs=[],
                            bass_nofuse=True,
                            sync_info=mybir.SyncInfo(
                                on_wait=excess[i : i + max_waits], on_update=[]
                            ),
                        )
                        nc.register_instruction(nop)
                        out.append(nop)
                    si.on_wait = keep
                out.append(inst)
            bb.instructions[:] = out


def _build_program():
    import concourse.bass as bass
    import concourse.tile as tile
    from concourse import mybir

    f32 = mybir.dt.float32
    mult = mybir.AluOpType.mult
    add = mybir.AluOpType.add
    subtract = mybir.AluOpType.subtract
    Copy = mybir.ActivationFunctionType.Copy
    BCAST32 = [0] * 32  # stream_shuffle: every lane reads lane 0 of its 32-group

    nc = bass.Bass()
    mem_ext = nc.dram_tensor("memory", [B_PER_CORE, H, D, D], f32, kind="ExternalInput")
    k_ext = nc.dram_tensor("k", [B_PER_CORE, H, D, 1], f32, kind="ExternalInput")
    v_ext = nc.dram_tensor("v", [B_PER_CORE, H, D, 1], f32, kind="ExternalInput")
    alpha_ext = nc.dram_tensor("alpha", [B_PER_CORE, H, 1, 1], f32, kind="ExternalInput")
    eta_ext = nc.dram_tensor("eta", [B_PER_CORE, H, 1, 1], f32, kind="ExternalInput")
    out_ext = nc.dram_tensor("out", [B_PER_CORE, H, D, D], f32, kind="ExternalOutput")

    NP = PAIRS_PER_CORE
    GD = G * C * D  # free elems per group tile (4096)
    p4 = slice(0, P, 32)  # partitions {0,32,64,96}

    with tile.TileContext(nc) as tc:
        with (
            tc.tile_pool(name="const", bufs=1) as const_pool,
            tc.tile_pool(name="m_in", bufs=4) as m_pool,
            tc.tile_pool(name="ms", bufs=2) as ms_pool,
            tc.tile_pool(name="outer", bufs=2) as outer_pool,
            tc.tile_pool(name="outt", bufs=3) as out_pool,
            tc.tile_pool(name="kbc", bufs=3) as kbc_pool,
            tc.tile_pool(name="scratch", bufs=2) as scratch_pool,
            tc.tile_pool(name="tmpp", bufs=2) as tmp_pool,
            tc.tile_pool(name="small", bufs=3) as small_pool,
        ):
            # --- one-time: broadcast alpha (cols 0:NP) and eta (cols NP:2NP) ---
            ab4 = const_pool.tile([P, 2 * NP], f32)
            nc.vector.memset(ab4[:], 0.0)
            nc.sync.dma_start(
                ab4[p4, 0:NP],
                alpha_ext[:].flatten().rearrange("(o c) -> o c", o=1).broadcast_to((4, NP)),
            )
            nc.sync.dma_start(
                ab4[p4, NP : 2 * NP],
                eta_ext[:].flatten().rearrange("(o c) -> o c", o=1).broadcast_to((4, NP)),
            )
            ab_bc = const_pool.tile([P, 2 * NP], f32)
            nc.vector.stream_shuffle(ab_bc[:], ab4[:], BCAST32)

            # Long-lived ping-pong landing pads for k: the per-group DMA
            # writes only partitions {0,32,64,96}; the shuffle reads a full
            # [128] AP, so the tiles must stay initialized across groups.
            kb4_tiles = []
            for i in range(2):
                t = const_pool.tile([P, G * D], f32, tag=f"kb4_{i}")
                nc.vector.memset(t[:], 0.0)
                kb4_tiles.append(t)

            # --- main loop over groups of G adjacent pairs ---
            for g in range(NG):
                p0 = g * G
                b, h = divmod(p0, H)  # pairs (b, h) .. (b, h+G-1)

                m_in = m_pool.tile([P, GD], f32)
                # SWDGE path: SP alone saturates issuing all DMAs
                # flat AP: each partition's 8KB is contiguous in DRAM ->
                # 1 descriptor per partition instead of 4
                nc.gpsimd.dma_start(
                    m_in[:],
                    mem_ext[b, h : h + G].flatten().rearrange("(p x) -> p x", p=P),
                )
                kb4 = kb4_tiles[g % 2]
                nc.sync.dma_start(
                    kb4[p4, :],
                    k_ext[b, h : h + G]
                    .flatten()
                    .rearrange("(o x) -> o x", o=1)
                    .broadcast_to((4, G * D)),
                )
                v_pc = small_pool.tile([P, G * C], f32, tag="v_pc")
                nc.sync.dma_start(
                    v_pc[:].rearrange("p (q c) -> p q c", q=G),
                    v_ext[b, h : h + G].flatten().rearrange("(q p c) -> p q c", q=G, p=P),
                )

                # k broadcast to all partitions: k_bc[p, q*D+j] = k_q[j]
                k_bc = kbc_pool.tile([P, G * D], f32)
                nc.vector.stream_shuffle(k_bc[:], kb4[:], BCAST32)

                # Ms = alpha * M  (per pair: per-partition scale AP).
                # Chunks 0-2 on ACT, chunk 3 on DVE (tensor_scalar runs in
                # 2x mode) to balance the two engines.
                ms = ms_pool.tile([P, GD], f32)
                for q in range(G):
                    base = q * C * D
                    nc.scalar.activation(
                        ms[:, base : base + 3 * D],
                        m_in[:, base : base + 3 * D],
                        Copy,
                        scale=ab_bc[:, p0 + q : p0 + q + 1],
                    )
                    nc.vector.tensor_scalar_mul(
                        ms[:, base + 3 * D : base + 4 * D],
                        m_in[:, base + 3 * D : base + 4 * D],
                        ab_bc[:, p0 + q : p0 + q + 1],
                    )

                # product = M (*) k, one batched op
                tmp = tmp_pool.tile([P, GD], f32)
                nc.vector.tensor_tensor(
                    tmp[:].rearrange("p (q c j) -> p q c j", q=G, c=C),
                    m_in[:].rearrange("p (q c j) -> p q c j", q=G, c=C),
                    k_bc[:]
                    .rearrange("p (q o j) -> p q o j", q=G, o=1)
                    .broadcast_to((P, G, C, D)),
                    mult,
                )

                # pred[:, q*C+c] = row-sum of product chunk (ACT accumulate)
                pred = small_pool.tile([P, G * C], f32, tag="pred")
                trash = scratch_pool.tile([P, D], f32, tag="trash")
                for qc in range(G * C):
                    nc.scalar.activation(
                        trash[:],
                        tmp[:, qc * D : (qc + 1) * D],
                        Copy,
                        accum_out=pred[:, qc : qc + 1],
                    )

                # s_err = eta * (pred - v)
                terr = small_pool.tile([P, G * C], f32, tag="terr")
                nc.vector.tensor_tensor(terr[:], pred[:], v_pc[:], subtract)
                s_err = small_pool.tile([P, G * C], f32, tag="s_err")
                for q in range(G):
                    nc.vector.tensor_scalar_mul(
                        s_err[:, q * C : (q + 1) * C],
                        terr[:, q * C : (q + 1) * C],
                        ab_bc[:, NP + p0 + q : NP + p0 + q + 1],
                    )

                # outer[:, q, c, :] = k_bc[:, q, :] * s_err[:, q*C+c]
                outer = outer_pool.tile([P, GD], f32)
                for q in range(G):
                    for c in range(C):
                        nc.vector.tensor_scalar_mul(
                            outer[:, (q * C + c) * D : (q * C + c + 1) * D],
                            k_bc[:, q * D : (q + 1) * D],
                            s_err[:, q * C + c : q * C + c + 1],
                        )

                # out = Ms + outer  (GPSIMD, one batched op)
                out_t = out_pool.tile([P, GD], f32)
                nc.gpsimd.tensor_tensor(out_t[:], ms[:], outer[:], add)

                nc.sync.dma_start(
                    out_ext[b, h : h + G].flatten().rearrange("(p x) -> p x", p=P),
                    out_t[:],
                )

    # Raw Bass (no Bacc.compile) skips the InstISA byte-encoding pass; without
    # it walrus fails with "ISA wrong length" on extended instructions.
    mybir.codegen_inst_isa_subclasses(nc)
    _legalize_sync_waits(nc, mybir)
    return nc


def _get_program():
    if "nc" not in _CACHE:
        _CACHE["nc"] = _build_program()
    return _CACHE["nc"]


def _run(in_maps, **kwargs):
    from concourse.bass_utils import run_bass_kernel_spmd

    nc = _get_program()
    return run_bass_kernel_spmd(nc, in_maps, list(range(N_CORES)), **kwargs)


def _make_in_maps(memory, k, v, alpha, eta):
    def prep(x):
        return np.ascontiguousarray(np.asarray(x, dtype=np.float32))

    memory, k, v, alpha, eta = map(prep, (memory, k, v, alpha, eta))
    in_maps = []
    for i in range(N_CORES):
        s = slice(i * B_PER_CORE, (i + 1) * B_PER_CORE)
        in_maps.append(
            {
                "memory": memory[s],
                "k": k[s],
                "v": v[s],
                "alpha": alpha[s],
                "eta": eta[s],
            }
        )
    return in_maps


def kernel(memory, k, v, alpha, eta):
    res = _run(_make_in_maps(memory, k, v, alpha, eta))
    return np.concatenate(
        [res.results[i]["out"] for i in range(N_CORES)], axis=0
    )
